# revision 25
# baseline (speedup 1.0000x reference)
"""AugmentedGeneEmbedding kernel for 8 TRN2 NeuronCores (Bass/Tile).

Math (per token t with gene g = idx[t]):
    id_vec  = id_table[g]                                  # [128]
    e       = gene_idx_to_esm_idx[g]
    valid   = (g < N_GENES) & (0 < e < V_ESM)
    seq     = valid ? esm_table[e] @ Wp + bp : 0           # [256]
    h       = concat([id_vec, tanh(gate) * seq])           # [384]
    y       = gelu(h @ W1 + b1) @ W2 + b2                  # [128]

Every factor depends only on the gene, so y[t] = Y[g(t)] for a per-gene
table Y.  The kernel dedups tokens to unique genes and computes Y once
per gene, then expands Y to tokens with one-hot selection matmuls (no
token gather at all):

  Host: fold Wc = tanh(g) * (Wp @ W1_bot)  (and cb = tanh(g) * bp @
      W1_bot, zero for this input); snake-assign unique genes to
      8 cores x G slot-groups of 128 slots each, balancing token counts
      (each group ends up with <=128 genes and <=512 tokens); build
      per-group one-hot SEL[slot, tok] matrices in bf16.  G is the
      smallest group count that fits the unique genes (19 here).
  Phase A (device, per gene tile of <=4 groups): gather esm+id rows
      (transposing SWDGE gather), z = Wc.T@esm + W1_top.T@id
      (+ mask*cb), a = gelu(z + b1), Y_q = a @ W2 + b2 -> SBUF bf16.
  Phase B (device, per 128-slot group, fused into phase A): one PE
      matmul out[feat, tok] = Y_q.T @ SEL_q, DVE copy to bf16, write
      out columns.  Runs immediately after each Y chunk; no DRAM
      round-trip, no phase-B gathers.

Startup is dominated by the fixed NEFF preamble (~6us) and the gpsimd
gather-ucode IRAM load (~6us), so all constant loads are coalesced into
4 HWDGE DMAs (idx / hot weights / f32 biases / SEL) to keep the rings
clear while the ucode loads, and the per-gather index-count register is
hoisted (one MOVE instead of ten).

SWDGE queue plan: gathers rotate through global DMASW sems in
scheduler-emission order; we build once with queue 0, read the emitted
sem rotation, rebuild with queue = sem % 4, verify, else fall back.
"""

import numpy as np
import ml_dtypes

N_CORES = 8
B, K = 32, 2048
N_GENES, ID_DIM, ESM_DIM, PROJ, V_ESM = 20000, 128, 1280, 256, 30000
NTOK_TOTAL = B * K

TPG = 512                         # token columns per slot-group (padded)
DUMMY_GATHER = False              # 16-idx IRAM-warm gather corrupts on HW
QUEUE_PLAN = True                 # rotate gathers across SWDGE queues

BF16 = ml_dtypes.bfloat16

_BUILD_CACHE = {}


def _tile_groups(n_groups):
    """Split n_groups slot-groups into gene tiles: a short 1,2 ramp for the
    earliest possible z start, then 4-group tiles (512-wide moving operands
    keep LDWEIGHTS hidden under the matmul stream)."""
    out = []
    g = 0
    while g < n_groups:
        if not out:
            ngrp = 1
        elif len(out) == 1:
            ngrp = min(2, n_groups - g)
        else:
            ngrp = min(4, n_groups - g)
        out.append(ngrp)
        g += ngrp
    return out


def build_nc(n_groups, has_cb, queue_plan=None):
    """Per-core Bass program (SPMD: same program on all 8 cores).
    queue_plan maps gather source-index -> SWDGE queue (default all 0).
    Gather source order: esm tile t -> 2t, id tile t -> 2t+1."""
    import concourse.bacc as bacc
    import concourse.mybir as mybir
    import concourse.tile as tile
    from concourse import library_config
    from contextlib import ExitStack

    fp32 = mybir.dt.float32
    bf16 = mybir.dt.bfloat16
    i16 = mybir.dt.int16
    AF = mybir.ActivationFunctionType

    ng_cap = n_groups * 128
    tiles = _tile_groups(n_groups)
    n_gt = len(tiles)
    qp = (queue_plan or {}).get
    W16 = ng_cap // 16
    HOT = 10 * PROJ + PROJ + 2 * ID_DIM    # wc | w1t | w2  (bf16 cols)

    nc = bacc.Bacc("TRN2", target_bir_lowering=False, num_swdge_queues=4)

    idx_d = nc.declare_dram_parameter("idx16", [128, 2 * W16], i16, isOutput=False)
    hot_d = nc.declare_dram_parameter("hotbf", [128, HOT], bf16, isOutput=False)
    f32_d = nc.declare_dram_parameter("f32w", [128, 130], fp32, isOutput=False)
    sel_d = nc.declare_dram_parameter("selbf", [128, n_groups * TPG], bf16, isOutput=False)
    esm_d = nc.declare_dram_parameter("esmbf", [V_ESM + 1, ESM_DIM], bf16, isOutput=False)
    id_d = nc.declare_dram_parameter("idbf", [N_GENES, ID_DIM], bf16, isOutput=False)
    if has_cb:
        mcb_d = nc.declare_dram_parameter("mcbbf", [1, ng_cap + PROJ], bf16, isOutput=False)
    # out column q*TPG + j holds token j of slot-group q (features on rows)
    out_d = nc.declare_dram_parameter("out", [128, n_groups * TPG], bf16, isOutput=True)

    with tile.TileContext(nc) as tc, ExitStack() as ctx:
        const = ctx.enter_context(tc.tile_pool(name="const", bufs=1))
        idp = ctx.enter_context(tc.tile_pool(name="idgat", bufs=n_gt))
        gpool = ctx.enter_context(tc.tile_pool(name="gather", bufs=n_gt))
        apool = ctx.enter_context(tc.tile_pool(name="act", bufs=4))
        ypool = ctx.enter_context(tc.tile_pool(name="ygrp", bufs=3))
        opool = ctx.enter_context(tc.tile_pool(name="tokout", bufs=3))
        zps = ctx.enter_context(tc.tile_pool(name="zps", bufs=3, space="PSUM"))
        yps = ctx.enter_context(tc.tile_pool(name="yps", bufs=2, space="PSUM"))
        bps = ctx.enter_context(tc.tile_pool(name="bps", bufs=3, space="PSUM"))

        # Gather ucode library load first, then a throwaway 16-row gather:
        # the first dma_gather on a freshly loaded library pays a ~6us IRAM
        # fetch, so burn it on a dummy while the index DMA is still in
        # flight.  The dummy's index tile is DVE-memset to zero (valid row 0
        # gathers) so it has no DMA dependency at all.
        nc.gpsimd.load_library(library_config.mlp)
        if DUMMY_GATHER:
            dum_idx = const.tile([128, 8], i16)
            nc.vector.memset(dum_idx[:], 0)
            dum_out = const.tile([128, 1, 128], bf16)
            nc.gpsimd.dma_gather(dum_out[:], id_d[:], dum_idx[:], 128,
                                 nc.gpsimd.compute_val(128), ID_DIM,
                                 transpose=True, queue_num=qp(0, 0))

        # Index blob is tile-major ([eidx|idid] per gene tile); tile 0's
        # slice loads as its own tiny DMA so the first gather — and with it
        # the lazy gather-ucode IRAM fetch — unblocks as early as possible.
        idx_sb = const.tile([128, 2 * W16], i16)
        c0 = 16 * tiles[0]
        nc.sync.dma_start(idx_sb[:, 0:c0], idx_d[:, 0:c0])
        nc.sync.dma_start(idx_sb[:, c0:], idx_d[:, c0:])

        # Warm the scalar-engine activation table set containing Gelu during
        # the preamble; otherwise the table load lands mid-stream and blocks
        # the scalar FIFO (and everything downstream) until all DMAs drain.
        f32_sb = const.tile([128, 130], fp32)
        nc.scalar.dma_start(f32_sb[:], f32_d[:])
        b1_sb = f32_sb[:, 0:2]
        b2b_sb = f32_sb[:, 2:130]
        warm = const.tile([128, 1], fp32)
        nc.scalar.activation(warm[:], b1_sb[:, 0:1], AF.Gelu, bias=b1_sb[:, 0:1])

        # Gathers for the whole gene table issued up front; ring backpressure
        # paces them.  esm before id per tile: the z chain consumes esm
        # chunks first, id only at the end.
        nreg = {}
        for gt in sorted(set(tiles)):
            nreg[gt] = nc.gpsimd.compute_val(gt * 128)
        gtiles = []
        itiles = []
        goff = 0
        for t, ngrp in enumerate(tiles):
            gn = ngrp * 128
            ic = goff * 16                 # tile-major idx blob
            gtile = gpool.tile([128, 10, gn], bf16, tag="G", name=f"G{t}")
            nc.gpsimd.dma_gather(gtile[:], esm_d[:],
                                 idx_sb[:, ic:ic + gn // 16], gn, nreg[ngrp],
                                 ESM_DIM, transpose=True,
                                 queue_num=qp(1 + 2 * t, 0))
            gtiles.append(gtile)
            itile = idp.tile([128, 1, gn], bf16, tag="I", name=f"I{t}")
            nc.gpsimd.dma_gather(itile[:], id_d[:],
                                 idx_sb[:, ic + gn // 16:ic + gn // 8], gn, nreg[ngrp],
                                 ID_DIM, transpose=True, queue_num=qp(2 + 2 * t, 0))
            itiles.append(itile)
            goff += ngrp

        # Weight loads after gather issuance in program order.
        hot_sb = const.tile([128, HOT], bf16)
        nc.sync.dma_start(hot_sb[:], hot_d[:])
        wc_sb = hot_sb[:, 0:10 * PROJ]               # [(c, f)] flat
        w1t_sb = hot_sb[:, 10 * PROJ:10 * PROJ + PROJ]
        w2_sb = hot_sb[:, 11 * PROJ:11 * PROJ + 2 * ID_DIM]
        if has_cb:
            mcb_sb = const.tile([1, ng_cap + PROJ], bf16)
            nc.scalar.dma_start(mcb_sb[:], mcb_d[:])
            mask_sb = mcb_sb[:, 0:ng_cap]
            cb_sb = mcb_sb[:, ng_cap:]
        # SEL loads per tile, each gated on that tile's esm gather DATA:
        # keeps the 2.5 MB SEL stream off the wire while the gather-ucode
        # IRAM fetch and the first gathers need it.  The gate is a pure data
        # dependency: a 1-element DVE copy of gather output into the SEL
        # slice makes the SEL DMA wait (WAW) for the gather (RAW).
        sel_sb = const.tile([128, n_groups * TPG], bf16)
        goff = 0
        for t, ngrp in enumerate(tiles):
            sl = slice(goff * TPG, (goff + ngrp) * TPG)
            nc.vector.tensor_copy(sel_sb[0:1, sl.start:sl.start + 1],
                                  gtiles[t][0:1, 0, 0:1])
            nc.scalar.dma_start(sel_sb[:, sl], sel_d[:, sl])
            goff += ngrp

        # ---------- fused phase A (per-gene Y) + phase B (token expand) ----
        goff = 0
        for t, ngrp in enumerate(tiles):
            gn = ngrp * 128
            gtile = gtiles[t]
            a_tiles = []
            for h in range(2):
                hs = slice(h * 128, (h + 1) * 128)
                zp = zps.tile([128, gn], fp32, tag="z", name=f"z{t}_{h}")
                for c in range(10):
                    nc.tensor.matmul(zp[:], wc_sb[:, c * PROJ + h * 128:
                                                  c * PROJ + h * 128 + 128],
                                     gtile[:, c, :], start=c == 0, stop=False)
                # id contribution late: each chain starts on esm data alone,
                # giving the (latency-bound) id gathers extra slack
                nc.tensor.matmul(zp[:], w1t_sb[:, hs], itiles[t][:, 0, :],
                                 start=False, stop=not has_cb)
                if has_cb:
                    nc.tensor.matmul(zp[:], cb_sb[0:1, hs],
                                     mask_sb[0:1, goff * 128:goff * 128 + gn],
                                     start=False, stop=True)
                at = apool.tile([128, gn], bf16, tag="a", name=f"a{t}_{h}")
                nc.scalar.activation(at[:], zp[:], AF.Gelu, bias=b1_sb[:, h:h + 1])
                a_tiles.append(at)
            osb = opool.tile([128, ngrp, TPG], bf16, tag="o", name=f"o{t}")
            for qq in range(ngrp):
                qs = slice(qq * 128, (qq + 1) * 128)
                yp = yps.tile([128, 128], fp32, tag="yp")
                nc.tensor.matmul(yp[:], a_tiles[0][:, qs], w2_sb[:, 0:ID_DIM],
                                 start=True, stop=False)
                nc.tensor.matmul(yp[:], a_tiles[1][:, qs], w2_sb[:, ID_DIM:],
                                 start=False, stop=True)
                yq = ypool.tile([128, 128], bf16, tag="y")
                nc.vector.tensor_add(yq[:], yp[:], b2b_sb[:])
                # phase B for this slot-group: one-hot selection matmul
                q = goff + qq
                bb = bps.tile([128, TPG], fp32, tag="b")
                nc.tensor.matmul(bb[:], yq[:], sel_sb[:, q * TPG:(q + 1) * TPG],
                                 start=True, stop=True)
                nc.vector.tensor_copy(osb[:, qq, :], bb[:])
            nc.sync.dma_start(out_d[:, goff * TPG:(goff + ngrp) * TPG],
                              osb[:].rearrange("p a b -> p (a b)"))
            goff += ngrp

    nc.compile()
    return nc


def _gather_emission(nc):
    """(num_idxs, elem_size, transpose, queue, sem_idx) per InstDMAGatherAnt
    in emission order."""
    import re
    out = []
    for i in nc.all_instructions():
        if type(i).__name__ != "InstDMAGatherAnt":
            continue
        sem = None
        if i.sync_info is not None:
            for u in i.sync_info.on_update:
                m = re.search(r"DMASW(\d+)_", str(u))
                if m:
                    sem = int(m.group(1))
        out.append((int(i.num_idxs), int(i.elem_size), bool(i.transpose),
                    int(i.queue_num), sem))
    return out


def _plan_queues(nc, n_groups):
    """Map gather source-index -> queue from the pass-1 sem rotation."""
    em = _gather_emission(nc)
    src = [(0, (128, ID_DIM, True))] if DUMMY_GATHER else []
    for t, ngrp in enumerate(_tile_groups(n_groups)):
        src.append((1 + 2 * t, (ngrp * 128, ESM_DIM, True)))
        src.append((2 + 2 * t, (ngrp * 128, ID_DIM, True)))
    if len(em) != len(src):
        return None
    from collections import defaultdict, deque
    pools = defaultdict(deque)
    for (ni, es, tr, q, sem) in em:
        if sem is None:
            return None
        pools[(ni, es, tr)].append(sem)
    plan = {}
    for si, sig in src:
        if not pools[sig]:
            return None
        plan[si] = pools[sig].popleft() % 4
    return plan


def _queues_consistent(nc):
    sems = {}
    for (ni, es, tr, q, sem) in _gather_emission(nc):
        if sem is None:
            return False
        if sems.setdefault(sem, q) != q:
            return False
    return True


def _build_best(n_groups, has_cb):
    nc0 = build_nc(n_groups, has_cb, None)
    if not QUEUE_PLAN:
        return nc0
    try:
        plan = _plan_queues(nc0, n_groups)
        if plan and any(q != 0 for q in plan.values()):
            nc1 = build_nc(n_groups, has_cb, plan)
            if _queues_consistent(nc1):
                return nc1
    except Exception:
        pass
    return nc0


def _wrap16(a16):
    """int16 [n] -> [128, n//16]: logical index i at [i % 16 (+16k), i // 16]."""
    w = a16.reshape(-1, 16).T
    return np.tile(w, (8, 1)).copy()


def _assign_bins(cnt, n_cores, n_groups):
    """Snake-assign genes (by count desc) to n_cores*n_groups bins.
    Returns (bin_of, ok): ok=False if any bin exceeds 128 genes or TPG
    tokens."""
    U = len(cnt)
    NB = n_cores * n_groups
    order = np.argsort(-cnt, kind="stable")
    k = np.arange(U)
    rnd = k // NB
    c = k % NB
    bin_snake = np.where(rnd % 2 == 0, c, NB - 1 - c)
    bin_of = np.empty(U, np.int64)
    bin_of[order] = bin_snake
    gcnt = np.bincount(bin_of, minlength=NB)
    tcnt = np.bincount(bin_of, weights=cnt, minlength=NB)
    return bin_of, bool(gcnt.max() <= 128 and tcnt.max() <= TPG)


def prepare_host(idx, gene_idx_to_esm_idx, id_table, esm_table, Wp, bp, gate,
                 W1, b1, W2, b2, n_cores=N_CORES):
    """Index prep, weight folding, dtype/layout marshalling.

    Returns (shared, per_core, tok_pos, n_groups, has_cb); tok_pos[c][q]
    are the original flat token positions in slot-group q of core c, in
    SEL column order."""
    idx_flat = np.asarray(idx).reshape(-1).astype(np.int64)
    gmap = np.asarray(gene_idx_to_esm_idx).astype(np.int64)
    g_clip = np.clip(idx_flat, 0, N_GENES - 1)
    oob = (idx_flat < 0) | (idx_flat >= N_GENES)
    # key encodes (id row, forced-invalid) so OOB tokens get mask=0 entries
    key = np.where(oob, g_clip + N_GENES, g_clip)
    uniq, inv = np.unique(key, return_inverse=True)
    U = len(uniq)
    cnt = np.bincount(inv, minlength=U)

    n_groups = -(-U // (128 * n_cores))
    bin_of, ok = _assign_bins(cnt, n_cores, n_groups)
    while not ok:
        n_groups += 1
        bin_of, ok = _assign_bins(cnt, n_cores, n_groups)
    NB = n_cores * n_groups
    ng_cap = n_groups * 128
    core_of = bin_of % n_cores
    grp_of = bin_of // n_cores
    # within each bin, order genes by key value (ascending table reads)
    rank_of = np.empty(U, np.int64)
    for b in range(NB):
        m = np.nonzero(bin_of == b)[0]        # ascending key order
        rank_of[m] = np.arange(len(m))
    slot_of = grp_of * 128 + rank_of

    urow = np.where(uniq >= N_GENES, uniq - N_GENES, uniq)   # id-table row
    ue = gmap[np.clip(urow, 0, N_GENES - 1)]
    uvalid = (uniq < N_GENES) & (ue > 0) & (ue < V_ESM)
    ueidx = np.where(uvalid, ue, V_ESM)                      # row V_ESM is zero pad

    eidx_core = np.full((n_cores, ng_cap), V_ESM, np.int16)
    idid_core = np.zeros((n_cores, ng_cap), np.int16)
    mask_core = np.zeros((n_cores, ng_cap), BF16)
    eidx_core[core_of, slot_of] = ueidx.astype(np.int16)
    idid_core[core_of, slot_of] = urow.astype(np.int16)
    mask_core[core_of, slot_of] = uvalid.astype(BF16)

    # tokens -> SEL one-hots: column j of (core, group) = j-th token of that
    # bin in flat order.  SEL stored partition-major: sel[p, q*TPG+j].
    tok_bin = bin_of[inv]
    tok_rank = rank_of[inv]
    bin_sort = np.argsort(tok_bin, kind="stable")  # flat positions by bin
    bcnt = np.bincount(tok_bin, minlength=NB)
    boff = np.concatenate([[0], np.cumsum(bcnt)])
    sel_core = np.zeros((n_cores, 128, n_groups * TPG), BF16)
    tok_pos = [[None] * n_groups for _ in range(n_cores)]
    for b in range(NB):
        pos = bin_sort[boff[b]:boff[b + 1]]
        cc, q = b % n_cores, b // n_cores
        tok_pos[cc][q] = pos
        sel_core[cc, tok_rank[pos], q * TPG + np.arange(len(pos))] = 1

    # host weight folding
    tg = np.tanh(float(np.asarray(gate).reshape(-1)[0]))
    Wp64 = np.asarray(Wp, np.float64)
    W1b = np.asarray(W1, np.float64)[ID_DIM:, :]
    Wc = tg * (Wp64 @ W1b)                                   # [1280, 256]
    cb = tg * (np.asarray(bp, np.float64) @ W1b)             # [256]
    has_cb = bool(np.abs(cb).max() > 1e-12)

    hot = np.empty((128, 10 * PROJ + PROJ + 2 * ID_DIM), BF16)
    hot[:, 0:10 * PROJ] = Wc.reshape(10, 128, PROJ).transpose(1, 0, 2) \
                            .reshape(128, 10 * PROJ).astype(BF16)
    hot[:, 10 * PROJ:11 * PROJ] = np.asarray(W1[:ID_DIM, :]).astype(BF16)
    hot[:, 11 * PROJ:] = np.asarray(W2).reshape(2, 128, ID_DIM) \
                           .transpose(1, 0, 2).reshape(128, 2 * ID_DIM).astype(BF16)
    f32w = np.empty((128, 130), np.float32)
    f32w[:, 0:2] = np.asarray(b1).astype(np.float32).reshape(2, 128).T
    f32w[:, 2:] = np.tile(np.asarray(b2).astype(np.float32).reshape(1, 128), (128, 1))

    shared = {
        "esmbf": np.concatenate(
            [np.asarray(esm_table).astype(BF16), np.zeros((1, ESM_DIM), BF16)], axis=0),
        "idbf": np.asarray(id_table).astype(BF16),
        "hotbf": hot,
        "f32w": f32w,
    }
    tiles = _tile_groups(n_groups)
    per_core = []
    for cc in range(n_cores):
        cols = []
        goff = 0
        for ngrp in tiles:
            s = slice(goff * 128, (goff + ngrp) * 128)
            cols.append(_wrap16(eidx_core[cc, s]))
            cols.append(_wrap16(idid_core[cc, s]))
            goff += ngrp
        pc = {
            "idx16": np.concatenate(cols, axis=1),
            "selbf": sel_core[cc],
        }
        if has_cb:
            pc["mcbbf"] = np.concatenate(
                [mask_core[cc], cb.astype(BF16)]).reshape(1, -1).copy()
        per_core.append(pc)
    return shared, per_core, tok_pos, n_groups, has_cb


def kernel(idx, gene_idx_to_esm_idx, id_table, esm_table, Wp, bp, gate,
           W1, b1, W2, b2, _trace=False, **_run_kwargs):
    from concourse.bass_utils import run_bass_kernel_spmd

    shared, per_core, tok_pos, n_groups, has_cb = prepare_host(
        idx, gene_idx_to_esm_idx, id_table, esm_table, Wp, bp, gate, W1, b1, W2, b2)
    bkey = (n_groups, has_cb)
    if bkey not in _BUILD_CACHE:
        _BUILD_CACHE[bkey] = _build_best(n_groups, has_cb)
    nc = _BUILD_CACHE[bkey]

    in_maps = [dict(shared, **pc) for pc in per_core]
    res = run_bass_kernel_spmd(nc, in_maps, list(range(N_CORES)), trace=_trace,
                               **_run_kwargs)
    sh = np.asarray(idx).shape
    out = np.empty((NTOK_TOTAL, ID_DIM), np.float32)
    for c in range(N_CORES):
        arr = np.asarray(res.results[c]["out"]).astype(np.float32)  # [128, G*TPG]
        for q in range(n_groups):
            pos = tok_pos[c][q]
            if len(pos):
                out[pos] = arr[:, q * TPG:q * TPG + len(pos)].T
    out = out.reshape(sh[0], sh[1], ID_DIM)
    if _trace:
        return out, res
    return out


# revision 28
# speedup vs baseline: 1.1080x; 1.1080x over previous
"""AugmentedGeneEmbedding kernel for 8 TRN2 NeuronCores (Bass/Tile).

Math (per token t with gene g = idx[t]):
    id_vec  = id_table[g]                                  # [128]
    e       = gene_idx_to_esm_idx[g]
    valid   = (g < N_GENES) & (0 < e < V_ESM)
    seq     = valid ? esm_table[e] @ Wp + bp : 0           # [256]
    h       = concat([id_vec, tanh(gate) * seq])           # [384]
    y       = gelu(h @ W1 + b1) @ W2 + b2                  # [128]

Every factor depends only on the gene, so y[t] = Y[g(t)] for a per-gene
table Y.  The kernel dedups tokens to unique genes and computes Y once
per gene, then expands Y to tokens with one-hot selection matmuls (no
token gather at all):

  Host: fold Wc = tanh(g) * (Wp @ W1_bot)  (and cb = tanh(g) * bp @
      W1_bot, zero for this input); snake-assign unique genes to
      8 cores x G slot-groups of 128 slots each, balancing token counts
      (each group ends up with <=128 genes and <=512 tokens); build
      per-group one-hot SEL[slot, tok] matrices in bf16.  G is the
      smallest group count that fits the unique genes (19 here).
  Phase A (device, per gene tile of <=4 groups): gather esm+id rows
      (transposing SWDGE gather), z = Wc.T@esm + W1_top.T@id
      (+ mask*cb), a = gelu(z + b1), Y_q = a @ W2 + b2 -> SBUF bf16.
  Phase B (device, per 128-slot group, fused into phase A): one PE
      matmul out[feat, tok] = Y_q.T @ SEL_q, DVE copy to bf16, write
      out columns.  Runs immediately after each Y chunk; no DRAM
      round-trip, no phase-B gathers.

Startup is dominated by the fixed NEFF preamble (~6us) and the gpsimd
gather-ucode IRAM load (~6us), so all constant loads are coalesced into
4 HWDGE DMAs (idx / hot weights / f32 biases / SEL) to keep the rings
clear while the ucode loads, and the per-gather index-count register is
hoisted (one MOVE instead of ten).

SWDGE queue plan: gathers rotate through global DMASW sems in
scheduler-emission order; we build once with queue 0, read the emitted
sem rotation, rebuild with queue = sem % 4, verify, else fall back.
"""

import numpy as np
import ml_dtypes

N_CORES = 8
B, K = 32, 2048
N_GENES, ID_DIM, ESM_DIM, PROJ, V_ESM = 20000, 128, 1280, 256, 30000
NTOK_TOTAL = B * K

TPG = 512                         # token columns per slot-group (padded)
DUMMY_GATHER = False              # 16-idx IRAM-warm gather corrupts on HW
QUEUE_PLAN = True                 # rotate gathers across SWDGE queues

BF16 = ml_dtypes.bfloat16

_BUILD_CACHE = {}


def _tile_groups(n_groups):
    """Split n_groups slot-groups into gene tiles: a short 1,2 ramp for the
    earliest possible z start, then 4-group tiles (512-wide moving operands
    keep LDWEIGHTS hidden under the matmul stream)."""
    out = []
    g = 0
    while g < n_groups:
        if not out:
            ngrp = 1
        elif len(out) == 1:
            ngrp = min(2, n_groups - g)
        else:
            ngrp = min(4, n_groups - g)
        out.append(ngrp)
        g += ngrp
    return out


def build_nc(n_groups, has_cb, queue_plan=None):
    """Per-core Bass program (SPMD: same program on all 8 cores).
    queue_plan maps gather source-index -> SWDGE queue (default all 0).
    Gather source order: esm tile t -> 2t, id tile t -> 2t+1."""
    import concourse.bacc as bacc
    import concourse.mybir as mybir
    import concourse.tile as tile
    from concourse import library_config
    from concourse.tile_rust import add_dep_helper
    from contextlib import ExitStack

    fp32 = mybir.dt.float32
    bf16 = mybir.dt.bfloat16
    i16 = mybir.dt.int16
    AF = mybir.ActivationFunctionType

    ng_cap = n_groups * 128
    tiles = _tile_groups(n_groups)
    n_gt = len(tiles)
    qp = (queue_plan or {}).get
    W16 = ng_cap // 16
    HOT = 10 * PROJ + PROJ + 2 * ID_DIM    # wc | w1t | w2  (bf16 cols)

    nc = bacc.Bacc("TRN2", target_bir_lowering=False, num_swdge_queues=4)

    idx_d = nc.declare_dram_parameter("idx16", [128, 2 * W16], i16, isOutput=False)
    hot_d = nc.declare_dram_parameter("hotbf", [128, HOT], bf16, isOutput=False)
    f32_d = nc.declare_dram_parameter("f32w", [128, 130], fp32, isOutput=False)
    sel_d = nc.declare_dram_parameter("selbf", [128, n_groups * TPG], bf16, isOutput=False)
    esm_d = nc.declare_dram_parameter("esmbf", [V_ESM + 1, ESM_DIM], bf16, isOutput=False)
    id_d = nc.declare_dram_parameter("idbf", [N_GENES, ID_DIM], bf16, isOutput=False)
    if has_cb:
        mcb_d = nc.declare_dram_parameter("mcbbf", [1, ng_cap + PROJ], bf16, isOutput=False)
    # out column q*TPG + j holds token j of slot-group q (features on rows)
    out_d = nc.declare_dram_parameter("out", [128, n_groups * TPG], bf16, isOutput=True)

    with tile.TileContext(nc) as tc, ExitStack() as ctx:
        const = ctx.enter_context(tc.tile_pool(name="const", bufs=1))
        idp = ctx.enter_context(tc.tile_pool(name="idgat", bufs=n_gt))
        gpool = ctx.enter_context(tc.tile_pool(name="gather", bufs=n_gt))
        apool = ctx.enter_context(tc.tile_pool(name="act", bufs=4))
        ypool = ctx.enter_context(tc.tile_pool(name="ygrp", bufs=3))
        opool = ctx.enter_context(tc.tile_pool(name="tokout", bufs=3))
        zps = ctx.enter_context(tc.tile_pool(name="zps", bufs=3, space="PSUM"))
        yps = ctx.enter_context(tc.tile_pool(name="yps", bufs=2, space="PSUM"))
        bps = ctx.enter_context(tc.tile_pool(name="bps", bufs=3, space="PSUM"))

        # Gather ucode library load first, then a throwaway 16-row gather:
        # the first dma_gather on a freshly loaded library pays a ~6us IRAM
        # fetch, so burn it on a dummy while the index DMA is still in
        # flight.  The dummy's index tile is DVE-memset to zero (valid row 0
        # gathers) so it has no DMA dependency at all.
        nc.gpsimd.load_library(library_config.mlp)
        if DUMMY_GATHER:
            dum_idx = const.tile([128, 8], i16)
            nc.vector.memset(dum_idx[:], 0)
            dum_out = const.tile([128, 1, 128], bf16)
            nc.gpsimd.dma_gather(dum_out[:], id_d[:], dum_idx[:], 128,
                                 nc.gpsimd.compute_val(128), ID_DIM,
                                 transpose=True, queue_num=qp(0, 0))

        # Index blob is tile-major ([eidx|idid] per gene tile); tile 0's
        # slice loads as its own tiny DMA so the first gather — and with it
        # the lazy gather-ucode IRAM fetch — unblocks as early as possible.
        idx_sb = const.tile([128, 2 * W16], i16)
        c0 = 16 * tiles[0]
        nc.sync.dma_start(idx_sb[:, 0:c0], idx_d[:, 0:c0])
        nc.sync.dma_start(idx_sb[:, c0:], idx_d[:, c0:])

        # Warm the scalar-engine activation table set containing Gelu during
        # the preamble; otherwise the table load lands mid-stream and blocks
        # the scalar FIFO (and everything downstream) until all DMAs drain.
        f32_sb = const.tile([128, 130], fp32)
        nc.scalar.dma_start(f32_sb[:], f32_d[:])
        b1_sb = f32_sb[:, 0:2]
        b2b_sb = f32_sb[:, 2:130]
        warm = const.tile([128, 1], fp32)
        nc.scalar.activation(warm[:], b1_sb[:, 0:1], AF.Gelu, bias=b1_sb[:, 0:1])

        # Gathers for the whole gene table issued up front; ring backpressure
        # paces them.  esm before id per tile: the z chain consumes esm
        # chunks first, id only at the end.
        nreg = {}
        for gt in sorted(set(tiles)):
            nreg[gt] = nc.gpsimd.compute_val(gt * 128)
        gtiles = []
        itiles = []
        esm_insts = []
        goff = 0
        for t, ngrp in enumerate(tiles):
            gn = ngrp * 128
            ic = goff * 16                 # tile-major idx blob
            gtile = gpool.tile([128, 10, gn], bf16, tag="G", name=f"G{t}")
            gi = nc.gpsimd.dma_gather(gtile[:], esm_d[:],
                                      idx_sb[:, ic:ic + gn // 16], gn, nreg[ngrp],
                                      ESM_DIM, transpose=True,
                                      queue_num=qp(1 + 2 * t, 0))
            esm_insts.append(gi)
            gtiles.append(gtile)
            itile = idp.tile([128, 1, gn], bf16, tag="I", name=f"I{t}")
            nc.gpsimd.dma_gather(itile[:], id_d[:],
                                 idx_sb[:, ic + gn // 16:ic + gn // 8], gn, nreg[ngrp],
                                 ID_DIM, transpose=True, queue_num=qp(2 + 2 * t, 0))
            itiles.append(itile)
            goff += ngrp

        # Weight loads after gather issuance in program order.
        hot_sb = const.tile([128, HOT], bf16)
        nc.sync.dma_start(hot_sb[:], hot_d[:])
        wc_sb = hot_sb[:, 0:10 * PROJ]               # [(c, f)] flat
        w1t_sb = hot_sb[:, 10 * PROJ:10 * PROJ + PROJ]
        w2_sb = hot_sb[:, 11 * PROJ:11 * PROJ + 2 * ID_DIM]
        if has_cb:
            mcb_sb = const.tile([1, ng_cap + PROJ], bf16)
            nc.scalar.dma_start(mcb_sb[:], mcb_d[:])
            mask_sb = mcb_sb[:, 0:ng_cap]
            cb_sb = mcb_sb[:, ng_cap:]
        # SEL loads per tile, each gated on a preceding esm gather's Q7
        # emission (NOT its data): keeps the 2.5 MB SEL stream off the wire
        # while the gather-ucode IRAM fetch and the first emissions need it,
        # without trailing the whole gather wire like a data dep would.
        sel_sb = const.tile([128, n_groups * TPG], bf16)
        goff = 0
        for t, ngrp in enumerate(tiles):
            sl = slice(goff * TPG, (goff + ngrp) * TPG)
            si = nc.scalar.dma_start(sel_sb[:, sl], sel_d[:, sl])
            add_dep_helper(si.ins, esm_insts[max(t - 1, 0)].ins, sync=True,
                           reason="sel load yields to gather stream")
            goff += ngrp

        # ---------- fused phase A (per-gene Y) + phase B (token expand) ----
        goff = 0
        for t, ngrp in enumerate(tiles):
            gn = ngrp * 128
            gtile = gtiles[t]
            a_tiles = []
            for h in range(2):
                hs = slice(h * 128, (h + 1) * 128)
                zp = zps.tile([128, gn], fp32, tag="z", name=f"z{t}_{h}")
                for c in range(10):
                    nc.tensor.matmul(zp[:], wc_sb[:, c * PROJ + h * 128:
                                                  c * PROJ + h * 128 + 128],
                                     gtile[:, c, :], start=c == 0, stop=False)
                # id contribution late: each chain starts on esm data alone,
                # giving the (latency-bound) id gathers extra slack
                nc.tensor.matmul(zp[:], w1t_sb[:, hs], itiles[t][:, 0, :],
                                 start=False, stop=not has_cb)
                if has_cb:
                    nc.tensor.matmul(zp[:], cb_sb[0:1, hs],
                                     mask_sb[0:1, goff * 128:goff * 128 + gn],
                                     start=False, stop=True)
                at = apool.tile([128, gn], bf16, tag="a", name=f"a{t}_{h}")
                nc.scalar.activation(at[:], zp[:], AF.Gelu, bias=b1_sb[:, h:h + 1])
                a_tiles.append(at)
            osb = opool.tile([128, ngrp, TPG], bf16, tag="o", name=f"o{t}")
            for qq in range(ngrp):
                qs = slice(qq * 128, (qq + 1) * 128)
                yp = yps.tile([128, 128], fp32, tag="yp")
                nc.tensor.matmul(yp[:], a_tiles[0][:, qs], w2_sb[:, 0:ID_DIM],
                                 start=True, stop=False)
                nc.tensor.matmul(yp[:], a_tiles[1][:, qs], w2_sb[:, ID_DIM:],
                                 start=False, stop=True)
                yq = ypool.tile([128, 128], bf16, tag="y")
                nc.vector.tensor_add(yq[:], yp[:], b2b_sb[:])
                # phase B for this slot-group: one-hot selection matmul
                q = goff + qq
                bb = bps.tile([128, TPG], fp32, tag="b")
                nc.tensor.matmul(bb[:], yq[:], sel_sb[:, q * TPG:(q + 1) * TPG],
                                 start=True, stop=True)
                nc.vector.tensor_copy(osb[:, qq, :], bb[:])
            nc.sync.dma_start(out_d[:, goff * TPG:(goff + ngrp) * TPG],
                              osb[:].rearrange("p a b -> p (a b)"))
            goff += ngrp

    nc.compile()
    return nc


def _gather_emission(nc):
    """(num_idxs, elem_size, transpose, queue, sem_idx) per InstDMAGatherAnt
    in emission order."""
    import re
    out = []
    for i in nc.all_instructions():
        if type(i).__name__ != "InstDMAGatherAnt":
            continue
        sem = None
        if i.sync_info is not None:
            for u in i.sync_info.on_update:
                m = re.search(r"DMASW(\d+)_", str(u))
                if m:
                    sem = int(m.group(1))
        out.append((int(i.num_idxs), int(i.elem_size), bool(i.transpose),
                    int(i.queue_num), sem))
    return out


def _plan_queues(nc, n_groups):
    """Map gather source-index -> queue from the pass-1 sem rotation."""
    em = _gather_emission(nc)
    src = [(0, (128, ID_DIM, True))] if DUMMY_GATHER else []
    for t, ngrp in enumerate(_tile_groups(n_groups)):
        src.append((1 + 2 * t, (ngrp * 128, ESM_DIM, True)))
        src.append((2 + 2 * t, (ngrp * 128, ID_DIM, True)))
    if len(em) != len(src):
        return None
    from collections import defaultdict, deque
    pools = defaultdict(deque)
    for (ni, es, tr, q, sem) in em:
        if sem is None:
            return None
        pools[(ni, es, tr)].append(sem)
    plan = {}
    for si, sig in src:
        if not pools[sig]:
            return None
        plan[si] = pools[sig].popleft() % 4
    return plan


def _queues_consistent(nc):
    sems = {}
    for (ni, es, tr, q, sem) in _gather_emission(nc):
        if sem is None:
            return False
        if sems.setdefault(sem, q) != q:
            return False
    return True


def _build_best(n_groups, has_cb):
    nc0 = build_nc(n_groups, has_cb, None)
    if not QUEUE_PLAN:
        return nc0
    try:
        plan = _plan_queues(nc0, n_groups)
        if plan and any(q != 0 for q in plan.values()):
            nc1 = build_nc(n_groups, has_cb, plan)
            if _queues_consistent(nc1):
                return nc1
    except Exception:
        pass
    return nc0


def _wrap16(a16):
    """int16 [n] -> [128, n//16]: logical index i at [i % 16 (+16k), i // 16]."""
    w = a16.reshape(-1, 16).T
    return np.tile(w, (8, 1)).copy()


def _assign_bins(cnt, n_cores, n_groups):
    """Snake-assign genes (by count desc) to n_cores*n_groups bins.
    Returns (bin_of, ok): ok=False if any bin exceeds 128 genes or TPG
    tokens."""
    U = len(cnt)
    NB = n_cores * n_groups
    order = np.argsort(-cnt, kind="stable")
    k = np.arange(U)
    rnd = k // NB
    c = k % NB
    bin_snake = np.where(rnd % 2 == 0, c, NB - 1 - c)
    bin_of = np.empty(U, np.int64)
    bin_of[order] = bin_snake
    gcnt = np.bincount(bin_of, minlength=NB)
    tcnt = np.bincount(bin_of, weights=cnt, minlength=NB)
    return bin_of, bool(gcnt.max() <= 128 and tcnt.max() <= TPG)


def prepare_host(idx, gene_idx_to_esm_idx, id_table, esm_table, Wp, bp, gate,
                 W1, b1, W2, b2, n_cores=N_CORES):
    """Index prep, weight folding, dtype/layout marshalling.

    Returns (shared, per_core, tok_pos, n_groups, has_cb); tok_pos[c][q]
    are the original flat token positions in slot-group q of core c, in
    SEL column order."""
    idx_flat = np.asarray(idx).reshape(-1).astype(np.int64)
    gmap = np.asarray(gene_idx_to_esm_idx).astype(np.int64)
    g_clip = np.clip(idx_flat, 0, N_GENES - 1)
    oob = (idx_flat < 0) | (idx_flat >= N_GENES)
    # key encodes (id row, forced-invalid) so OOB tokens get mask=0 entries
    key = np.where(oob, g_clip + N_GENES, g_clip)
    uniq, inv = np.unique(key, return_inverse=True)
    U = len(uniq)
    cnt = np.bincount(inv, minlength=U)

    n_groups = -(-U // (128 * n_cores))
    bin_of, ok = _assign_bins(cnt, n_cores, n_groups)
    while not ok:
        n_groups += 1
        bin_of, ok = _assign_bins(cnt, n_cores, n_groups)
    NB = n_cores * n_groups
    ng_cap = n_groups * 128
    core_of = bin_of % n_cores
    grp_of = bin_of // n_cores
    # within each bin, order genes by key value (ascending table reads)
    rank_of = np.empty(U, np.int64)
    for b in range(NB):
        m = np.nonzero(bin_of == b)[0]        # ascending key order
        rank_of[m] = np.arange(len(m))
    slot_of = grp_of * 128 + rank_of

    urow = np.where(uniq >= N_GENES, uniq - N_GENES, uniq)   # id-table row
    ue = gmap[np.clip(urow, 0, N_GENES - 1)]
    uvalid = (uniq < N_GENES) & (ue > 0) & (ue < V_ESM)
    ueidx = np.where(uvalid, ue, V_ESM)                      # row V_ESM is zero pad

    eidx_core = np.full((n_cores, ng_cap), V_ESM, np.int16)
    idid_core = np.zeros((n_cores, ng_cap), np.int16)
    mask_core = np.zeros((n_cores, ng_cap), BF16)
    eidx_core[core_of, slot_of] = ueidx.astype(np.int16)
    idid_core[core_of, slot_of] = urow.astype(np.int16)
    mask_core[core_of, slot_of] = uvalid.astype(BF16)

    # tokens -> SEL one-hots: column j of (core, group) = j-th token of that
    # bin in flat order.  SEL stored partition-major: sel[p, q*TPG+j].
    tok_bin = bin_of[inv]
    tok_rank = rank_of[inv]
    bin_sort = np.argsort(tok_bin, kind="stable")  # flat positions by bin
    bcnt = np.bincount(tok_bin, minlength=NB)
    boff = np.concatenate([[0], np.cumsum(bcnt)])
    sel_core = np.zeros((n_cores, 128, n_groups * TPG), BF16)
    tok_pos = [[None] * n_groups for _ in range(n_cores)]
    for b in range(NB):
        pos = bin_sort[boff[b]:boff[b + 1]]
        cc, q = b % n_cores, b // n_cores
        tok_pos[cc][q] = pos
        sel_core[cc, tok_rank[pos], q * TPG + np.arange(len(pos))] = 1

    # host weight folding
    tg = np.tanh(float(np.asarray(gate).reshape(-1)[0]))
    Wp64 = np.asarray(Wp, np.float64)
    W1b = np.asarray(W1, np.float64)[ID_DIM:, :]
    Wc = tg * (Wp64 @ W1b)                                   # [1280, 256]
    cb = tg * (np.asarray(bp, np.float64) @ W1b)             # [256]
    has_cb = bool(np.abs(cb).max() > 1e-12)

    hot = np.empty((128, 10 * PROJ + PROJ + 2 * ID_DIM), BF16)
    hot[:, 0:10 * PROJ] = Wc.reshape(10, 128, PROJ).transpose(1, 0, 2) \
                            .reshape(128, 10 * PROJ).astype(BF16)
    hot[:, 10 * PROJ:11 * PROJ] = np.asarray(W1[:ID_DIM, :]).astype(BF16)
    hot[:, 11 * PROJ:] = np.asarray(W2).reshape(2, 128, ID_DIM) \
                           .transpose(1, 0, 2).reshape(128, 2 * ID_DIM).astype(BF16)
    f32w = np.empty((128, 130), np.float32)
    f32w[:, 0:2] = np.asarray(b1).astype(np.float32).reshape(2, 128).T
    f32w[:, 2:] = np.tile(np.asarray(b2).astype(np.float32).reshape(1, 128), (128, 1))

    shared = {
        "esmbf": np.concatenate(
            [np.asarray(esm_table).astype(BF16), np.zeros((1, ESM_DIM), BF16)], axis=0),
        "idbf": np.asarray(id_table).astype(BF16),
        "hotbf": hot,
        "f32w": f32w,
    }
    tiles = _tile_groups(n_groups)
    per_core = []
    for cc in range(n_cores):
        cols = []
        goff = 0
        for ngrp in tiles:
            s = slice(goff * 128, (goff + ngrp) * 128)
            cols.append(_wrap16(eidx_core[cc, s]))
            cols.append(_wrap16(idid_core[cc, s]))
            goff += ngrp
        pc = {
            "idx16": np.concatenate(cols, axis=1),
            "selbf": sel_core[cc],
        }
        if has_cb:
            pc["mcbbf"] = np.concatenate(
                [mask_core[cc], cb.astype(BF16)]).reshape(1, -1).copy()
        per_core.append(pc)
    return shared, per_core, tok_pos, n_groups, has_cb


def kernel(idx, gene_idx_to_esm_idx, id_table, esm_table, Wp, bp, gate,
           W1, b1, W2, b2, _trace=False, **_run_kwargs):
    from concourse.bass_utils import run_bass_kernel_spmd

    shared, per_core, tok_pos, n_groups, has_cb = prepare_host(
        idx, gene_idx_to_esm_idx, id_table, esm_table, Wp, bp, gate, W1, b1, W2, b2)
    bkey = (n_groups, has_cb)
    if bkey not in _BUILD_CACHE:
        _BUILD_CACHE[bkey] = _build_best(n_groups, has_cb)
    nc = _BUILD_CACHE[bkey]

    in_maps = [dict(shared, **pc) for pc in per_core]
    res = run_bass_kernel_spmd(nc, in_maps, list(range(N_CORES)), trace=_trace,
                               **_run_kwargs)
    sh = np.asarray(idx).shape
    out = np.empty((NTOK_TOTAL, ID_DIM), np.float32)
    for c in range(N_CORES):
        arr = np.asarray(res.results[c]["out"]).astype(np.float32)  # [128, G*TPG]
        for q in range(n_groups):
            pos = tok_pos[c][q]
            if len(pos):
                out[pos] = arr[:, q * TPG:q * TPG + len(pos)].T
    out = out.reshape(sh[0], sh[1], ID_DIM)
    if _trace:
        return out, res
    return out


# revision 33
# speedup vs baseline: 1.1084x; 1.0003x over previous
"""AugmentedGeneEmbedding kernel for 8 TRN2 NeuronCores (Bass/Tile).

Math (per token t with gene g = idx[t]):
    id_vec  = id_table[g]                                  # [128]
    e       = gene_idx_to_esm_idx[g]
    valid   = (g < N_GENES) & (0 < e < V_ESM)
    seq     = valid ? esm_table[e] @ Wp + bp : 0           # [256]
    h       = concat([id_vec, tanh(gate) * seq])           # [384]
    y       = gelu(h @ W1 + b1) @ W2 + b2                  # [128]

Every factor depends only on the gene, so y[t] = Y[g(t)] for a per-gene
table Y.  The kernel dedups tokens to unique genes and computes Y once
per gene, then expands Y to tokens with one-hot selection matmuls (no
token gather at all):

  Host: fold Wc = tanh(g) * (Wp @ W1_bot)  (and cb = tanh(g) * bp @
      W1_bot, zero for this input); snake-assign unique genes to
      8 cores x G slot-groups of 128 slots each, balancing token counts
      (each group ends up with <=128 genes and <=512 tokens); build
      per-group one-hot SEL[slot, tok] matrices in bf16.  G is the
      smallest group count that fits the unique genes (19 here).
  Phase A (device, per gene tile of <=4 groups): gather esm+id rows
      (transposing SWDGE gather), z = Wc.T@esm + W1_top.T@id
      (+ mask*cb), a = gelu(z + b1), Y_q = a @ W2 + b2 -> SBUF bf16.
  Phase B (device, per 128-slot group, fused into phase A): one PE
      matmul out[feat, tok] = Y_q.T @ SEL_q, DVE copy to bf16, write
      out columns.  Runs immediately after each Y chunk; no DRAM
      round-trip, no phase-B gathers.

Startup is dominated by the fixed NEFF preamble (~6us) and the gpsimd
gather-ucode IRAM load (~6us), so all constant loads are coalesced into
4 HWDGE DMAs (idx / hot weights / f32 biases / SEL) to keep the rings
clear while the ucode loads, and the per-gather index-count register is
hoisted (one MOVE instead of ten).

SWDGE queue plan: gathers rotate through global DMASW sems in
scheduler-emission order; we build once with queue 0, read the emitted
sem rotation, rebuild with queue = sem % 4, verify, else fall back.
"""

import numpy as np
import ml_dtypes

N_CORES = 8
B, K = 32, 2048
N_GENES, ID_DIM, ESM_DIM, PROJ, V_ESM = 20000, 128, 1280, 256, 30000
NTOK_TOTAL = B * K

TPG = 512                         # token columns per slot-group (padded)
DUMMY_GATHER = False              # 16-idx IRAM-warm gather corrupts on HW
QUEUE_PLAN = True                 # rotate gathers across SWDGE queues

BF16 = ml_dtypes.bfloat16

_BUILD_CACHE = {}


def _tile_groups(n_groups):
    """Split n_groups slot-groups into gene tiles: a short 1,2 ramp for the
    earliest possible z start, then 4-group tiles (512-wide moving operands
    keep LDWEIGHTS hidden under the matmul stream)."""
    out = []
    g = 0
    while g < n_groups:
        if not out:
            ngrp = 1
        elif len(out) == 1:
            ngrp = min(2, n_groups - g)
        else:
            ngrp = min(4, n_groups - g)
        out.append(ngrp)
        g += ngrp
    return out


def build_nc(n_groups, tpg, has_cb, queue_plan=None):
    """Per-core Bass program (SPMD: same program on all 8 cores).
    queue_plan maps gather source-index -> SWDGE queue (default all 0).
    Gather source order: esm tile t -> 2t, id tile t -> 2t+1."""
    import concourse.bacc as bacc
    import concourse.mybir as mybir
    import concourse.tile as tile
    from concourse import library_config
    from concourse.tile_rust import add_dep_helper
    from contextlib import ExitStack

    fp32 = mybir.dt.float32
    bf16 = mybir.dt.bfloat16
    i16 = mybir.dt.int16
    AF = mybir.ActivationFunctionType

    TPG = tpg
    ng_cap = n_groups * 128
    tiles = _tile_groups(n_groups)
    n_gt = len(tiles)
    qp = (queue_plan or {}).get
    W16 = ng_cap // 16
    HOT = 10 * PROJ + PROJ + 2 * ID_DIM    # wc | w1t | w2  (bf16 cols)

    nc = bacc.Bacc("TRN2", target_bir_lowering=False, num_swdge_queues=4)

    idx_d = nc.declare_dram_parameter("idx16", [128, 2 * W16], i16, isOutput=False)
    hot_d = nc.declare_dram_parameter("hotbf", [128, HOT], bf16, isOutput=False)
    f32_d = nc.declare_dram_parameter("f32w", [128, 130], fp32, isOutput=False)
    sel_d = nc.declare_dram_parameter("selbf", [128, n_groups * TPG], bf16, isOutput=False)
    esm_d = nc.declare_dram_parameter("esmbf", [V_ESM + 1, ESM_DIM], bf16, isOutput=False)
    id_d = nc.declare_dram_parameter("idbf", [N_GENES, ID_DIM], bf16, isOutput=False)
    if has_cb:
        mcb_d = nc.declare_dram_parameter("mcbbf", [1, ng_cap + PROJ], bf16, isOutput=False)
    # out column q*TPG + j holds token j of slot-group q (features on rows)
    out_d = nc.declare_dram_parameter("out", [128, n_groups * TPG], bf16, isOutput=True)

    with tile.TileContext(nc) as tc, ExitStack() as ctx:
        const = ctx.enter_context(tc.tile_pool(name="const", bufs=1))
        idp = ctx.enter_context(tc.tile_pool(name="idgat", bufs=n_gt))
        gpool = ctx.enter_context(tc.tile_pool(name="gather", bufs=n_gt))
        apool = ctx.enter_context(tc.tile_pool(name="act", bufs=4))
        ypool = ctx.enter_context(tc.tile_pool(name="ygrp", bufs=10))
        opool = ctx.enter_context(tc.tile_pool(name="tokout", bufs=3))
        zps = ctx.enter_context(tc.tile_pool(name="zps", bufs=3, space="PSUM"))
        yps = ctx.enter_context(tc.tile_pool(name="yps", bufs=2, space="PSUM"))
        bps = ctx.enter_context(tc.tile_pool(name="bps", bufs=3, space="PSUM"))

        # Gather ucode library load first, then a throwaway 16-row gather:
        # the first dma_gather on a freshly loaded library pays a ~6us IRAM
        # fetch, so burn it on a dummy while the index DMA is still in
        # flight.  The dummy's index tile is DVE-memset to zero (valid row 0
        # gathers) so it has no DMA dependency at all.
        nc.gpsimd.load_library(library_config.mlp)
        if DUMMY_GATHER:
            dum_idx = const.tile([128, 8], i16)
            nc.vector.memset(dum_idx[:], 0)
            dum_out = const.tile([128, 1, 128], bf16)
            nc.gpsimd.dma_gather(dum_out[:], id_d[:], dum_idx[:], 128,
                                 nc.gpsimd.compute_val(128), ID_DIM,
                                 transpose=True, queue_num=qp(0, 0))

        # Index blob is tile-major ([eidx|idid] per gene tile); tile 0's
        # slice loads as its own tiny DMA so the first gather — and with it
        # the lazy gather-ucode IRAM fetch — unblocks as early as possible.
        idx_sb = const.tile([128, 2 * W16], i16)
        c0 = 16 * tiles[0]
        nc.sync.dma_start(idx_sb[:, 0:c0], idx_d[:, 0:c0])
        nc.sync.dma_start(idx_sb[:, c0:], idx_d[:, c0:])

        # Warm the scalar-engine activation table set containing Gelu during
        # the preamble; otherwise the table load lands mid-stream and blocks
        # the scalar FIFO (and everything downstream) until all DMAs drain.
        f32_sb = const.tile([128, 130], fp32)
        nc.scalar.dma_start(f32_sb[:], f32_d[:])
        b1_sb = f32_sb[:, 0:2]
        b2b_sb = f32_sb[:, 2:130]
        warm = const.tile([128, 1], fp32)
        nc.scalar.activation(warm[:], b1_sb[:, 0:1], AF.Gelu, bias=b1_sb[:, 0:1])

        # Gathers for the whole gene table issued up front; ring backpressure
        # paces them.  esm before id per tile: the z chain consumes esm
        # chunks first, id only at the end.
        nreg = {}
        for gt in sorted(set(tiles)):
            nreg[gt] = nc.gpsimd.compute_val(gt * 128)
        gtiles = []
        itiles = []
        esm_insts = []
        goff = 0
        for t, ngrp in enumerate(tiles):
            gn = ngrp * 128
            ic = goff * 16                 # tile-major idx blob
            gtile = gpool.tile([128, 10, gn], bf16, tag="G", name=f"G{t}")
            gi = nc.gpsimd.dma_gather(gtile[:], esm_d[:],
                                      idx_sb[:, ic:ic + gn // 16], gn, nreg[ngrp],
                                      ESM_DIM, transpose=True,
                                      queue_num=qp(1 + 2 * t, 0))
            esm_insts.append(gi)
            gtiles.append(gtile)
            itile = idp.tile([128, 1, gn], bf16, tag="I", name=f"I{t}")
            nc.gpsimd.dma_gather(itile[:], id_d[:],
                                 idx_sb[:, ic + gn // 16:ic + gn // 8], gn, nreg[ngrp],
                                 ID_DIM, transpose=True,
                                 queue_num=qp(2 + 2 * t, 0))
            itiles.append(itile)
            goff += ngrp

        # Weight loads after gather issuance in program order.
        hot_sb = const.tile([128, HOT], bf16)
        nc.sync.dma_start(hot_sb[:], hot_d[:])
        wc_sb = hot_sb[:, 0:10 * PROJ]               # [(c, f)] flat
        w1t_sb = hot_sb[:, 10 * PROJ:10 * PROJ + PROJ]
        w2_sb = hot_sb[:, 11 * PROJ:11 * PROJ + 2 * ID_DIM]
        if has_cb:
            mcb_sb = const.tile([1, ng_cap + PROJ], bf16)
            nc.scalar.dma_start(mcb_sb[:], mcb_d[:])
            mask_sb = mcb_sb[:, 0:ng_cap]
            cb_sb = mcb_sb[:, ng_cap:]
        # SEL loads per tile, each gated on a preceding esm gather's Q7
        # emission (NOT its data): keeps the 2.5 MB SEL stream off the wire
        # while the gather-ucode IRAM fetch and the first emissions need it,
        # without trailing the whole gather wire like a data dep would.
        sel_sb = const.tile([128, n_groups * TPG], bf16)
        goff = 0
        for t, ngrp in enumerate(tiles):
            sl = slice(goff * TPG, (goff + ngrp) * TPG)
            si = nc.scalar.dma_start(sel_sb[:, sl], sel_d[:, sl])
            add_dep_helper(si.ins, esm_insts[max(t - 1, 0)].ins, sync=True,
                           reason="sel load yields to gather stream")
            goff += ngrp

        # ---------- fused phase A (per-gene Y) + phase B (token expand) ----
        # Phase B for tile t-1 is emitted AFTER tile t's z/y matmuls: the PE
        # queue is strict FIFO, so a B matmul whose SEL slice hasn't landed
        # yet must not sit in front of the next tile's (data-ready) z chain.
        def emit_B(pend):
            goff_, ngrp_, yqs = pend
            osb = opool.tile([128, ngrp_, TPG], bf16, tag="o", name=f"o{goff_}")
            for qq in range(ngrp_):
                q = goff_ + qq
                bb = bps.tile([128, TPG], fp32, tag="b")
                nc.tensor.matmul(bb[:], yqs[qq][:],
                                 sel_sb[:, q * TPG:(q + 1) * TPG],
                                 start=True, stop=True)
                nc.vector.tensor_copy(osb[:, qq, :], bb[:])
            nc.sync.dma_start(out_d[:, goff_ * TPG:(goff_ + ngrp_) * TPG],
                              osb[:].rearrange("p a b -> p (a b)"))

        pending = None
        goff = 0
        for t, ngrp in enumerate(tiles):
            gn = ngrp * 128
            gtile = gtiles[t]
            a_tiles = []
            for h in range(2):
                hs = slice(h * 128, (h + 1) * 128)
                zp = zps.tile([128, gn], fp32, tag="z", name=f"z{t}_{h}")
                for c in range(10):
                    nc.tensor.matmul(zp[:], wc_sb[:, c * PROJ + h * 128:
                                                  c * PROJ + h * 128 + 128],
                                     gtile[:, c, :], start=c == 0, stop=False)
                # id contribution late: each chain starts on esm data alone,
                # giving the (latency-bound) id gathers extra slack
                nc.tensor.matmul(zp[:], w1t_sb[:, hs], itiles[t][:, 0, :],
                                 start=False, stop=not has_cb)
                if has_cb:
                    nc.tensor.matmul(zp[:], cb_sb[0:1, hs],
                                     mask_sb[0:1, goff * 128:goff * 128 + gn],
                                     start=False, stop=True)
                at = apool.tile([128, gn], bf16, tag="a", name=f"a{t}_{h}")
                nc.scalar.activation(at[:], zp[:], AF.Gelu, bias=b1_sb[:, h:h + 1])
                a_tiles.append(at)
            yqs = []
            for qq in range(ngrp):
                qs = slice(qq * 128, (qq + 1) * 128)
                yp = yps.tile([128, 128], fp32, tag="yp")
                nc.tensor.matmul(yp[:], a_tiles[0][:, qs], w2_sb[:, 0:ID_DIM],
                                 start=True, stop=False)
                nc.tensor.matmul(yp[:], a_tiles[1][:, qs], w2_sb[:, ID_DIM:],
                                 start=False, stop=True)
                yq = ypool.tile([128, 128], bf16, tag="y")
                nc.vector.tensor_add(yq[:], yp[:], b2b_sb[:])
                yqs.append(yq)
            if pending is not None:
                emit_B(pending)
            pending = (goff, ngrp, yqs)
            goff += ngrp
        emit_B(pending)

    nc.compile()
    return nc


def _gather_emission(nc):
    """(num_idxs, elem_size, transpose, queue, sem_idx) per InstDMAGatherAnt
    in emission order."""
    import re
    out = []
    for i in nc.all_instructions():
        if type(i).__name__ != "InstDMAGatherAnt":
            continue
        sem = None
        if i.sync_info is not None:
            for u in i.sync_info.on_update:
                m = re.search(r"DMASW(\d+)_", str(u))
                if m:
                    sem = int(m.group(1))
        out.append((int(i.num_idxs), int(i.elem_size), bool(i.transpose),
                    int(i.queue_num), sem))
    return out


def _plan_queues(nc, n_groups):
    """Map gather source-index -> queue from the pass-1 sem rotation."""
    em = _gather_emission(nc)
    src = [(0, (128, ID_DIM, True))] if DUMMY_GATHER else []
    for t, ngrp in enumerate(_tile_groups(n_groups)):
        src.append((1 + 2 * t, (ngrp * 128, ESM_DIM, True)))
        src.append((2 + 2 * t, (ngrp * 128, ID_DIM, True)))
    if len(em) != len(src):
        return None
    from collections import defaultdict, deque
    pools = defaultdict(deque)
    for (ni, es, tr, q, sem) in em:
        if sem is None:
            return None
        pools[(ni, es, tr)].append(sem)
    plan = {}
    for si, sig in src:
        if not pools[sig]:
            return None
        plan[si] = pools[sig].popleft() % 4
    return plan


def _queues_consistent(nc):
    sems = {}
    for (ni, es, tr, q, sem) in _gather_emission(nc):
        if sem is None:
            return False
        if sems.setdefault(sem, q) != q:
            return False
    return True


def _build_best(n_groups, tpg, has_cb):
    nc0 = build_nc(n_groups, tpg, has_cb, None)
    if not QUEUE_PLAN:
        return nc0
    try:
        plan = _plan_queues(nc0, n_groups)
        if plan and any(q != 0 for q in plan.values()):
            nc1 = build_nc(n_groups, tpg, has_cb, plan)
            if _queues_consistent(nc1):
                return nc1
    except Exception:
        pass
    return nc0


def _wrap16(a16):
    """int16 [n] -> [128, n//16]: logical index i at [i % 16 (+16k), i // 16]."""
    w = a16.reshape(-1, 16).T
    return np.tile(w, (8, 1)).copy()


def _assign_bins(cnt, n_cores, n_groups):
    """Snake-assign genes (by count desc) to n_cores*n_groups bins.
    Returns (bin_of, ok): ok=False if any bin exceeds 128 genes or TPG
    tokens."""
    U = len(cnt)
    NB = n_cores * n_groups
    order = np.argsort(-cnt, kind="stable")
    k = np.arange(U)
    rnd = k // NB
    c = k % NB
    bin_snake = np.where(rnd % 2 == 0, c, NB - 1 - c)
    bin_of = np.empty(U, np.int64)
    bin_of[order] = bin_snake
    gcnt = np.bincount(bin_of, minlength=NB)
    tcnt = np.bincount(bin_of, weights=cnt, minlength=NB)
    return bin_of, bool(gcnt.max() <= 128 and tcnt.max() <= TPG)


def prepare_host(idx, gene_idx_to_esm_idx, id_table, esm_table, Wp, bp, gate,
                 W1, b1, W2, b2, n_cores=N_CORES):
    """Index prep, weight folding, dtype/layout marshalling.

    Returns (shared, per_core, tok_pos, n_groups, tpg, has_cb); tok_pos[c][q]
    are the original flat token positions in slot-group q of core c, in
    SEL column order."""
    idx_flat = np.asarray(idx).reshape(-1).astype(np.int64)
    gmap = np.asarray(gene_idx_to_esm_idx).astype(np.int64)
    g_clip = np.clip(idx_flat, 0, N_GENES - 1)
    oob = (idx_flat < 0) | (idx_flat >= N_GENES)
    # key encodes (id row, forced-invalid) so OOB tokens get mask=0 entries
    key = np.where(oob, g_clip + N_GENES, g_clip)
    uniq, inv = np.unique(key, return_inverse=True)
    U = len(uniq)
    cnt = np.bincount(inv, minlength=U)

    n_groups = -(-U // (128 * n_cores))
    bin_of, ok = _assign_bins(cnt, n_cores, n_groups)
    while not ok:
        n_groups += 1
        bin_of, ok = _assign_bins(cnt, n_cores, n_groups)
    NB = n_cores * n_groups
    ng_cap = n_groups * 128
    core_of = bin_of % n_cores
    grp_of = bin_of // n_cores
    # within each bin, order genes by key value (ascending table reads)
    rank_of = np.empty(U, np.int64)
    for b in range(NB):
        m = np.nonzero(bin_of == b)[0]        # ascending key order
        rank_of[m] = np.arange(len(m))
    slot_of = grp_of * 128 + rank_of

    urow = np.where(uniq >= N_GENES, uniq - N_GENES, uniq)   # id-table row
    ue = gmap[np.clip(urow, 0, N_GENES - 1)]
    uvalid = (uniq < N_GENES) & (ue > 0) & (ue < V_ESM)
    ueidx = np.where(uvalid, ue, V_ESM)                      # row V_ESM is zero pad

    eidx_core = np.full((n_cores, ng_cap), V_ESM, np.int16)
    idid_core = np.zeros((n_cores, ng_cap), np.int16)
    mask_core = np.zeros((n_cores, ng_cap), BF16)
    eidx_core[core_of, slot_of] = ueidx.astype(np.int16)
    idid_core[core_of, slot_of] = urow.astype(np.int16)
    mask_core[core_of, slot_of] = uvalid.astype(BF16)

    # tokens -> SEL one-hots: column j of (core, group) = j-th token of that
    # bin in flat order.  SEL stored partition-major: sel[p, q*TPG+j].
    tok_bin = bin_of[inv]
    tok_rank = rank_of[inv]
    bin_sort = np.argsort(tok_bin, kind="stable")  # flat positions by bin
    bcnt = np.bincount(tok_bin, minlength=NB)
    boff = np.concatenate([[0], np.cumsum(bcnt)])
    tpg = min(TPG, int(-(-int(bcnt.max()) // 64) * 64))   # padded col count
    sel_core = np.zeros((n_cores, 128, n_groups * tpg), BF16)
    tok_pos = [[None] * n_groups for _ in range(n_cores)]
    for b in range(NB):
        pos = bin_sort[boff[b]:boff[b + 1]]
        cc, q = b % n_cores, b // n_cores
        tok_pos[cc][q] = pos
        sel_core[cc, tok_rank[pos], q * tpg + np.arange(len(pos))] = 1

    # host weight folding
    tg = np.tanh(float(np.asarray(gate).reshape(-1)[0]))
    Wp64 = np.asarray(Wp, np.float64)
    W1b = np.asarray(W1, np.float64)[ID_DIM:, :]
    Wc = tg * (Wp64 @ W1b)                                   # [1280, 256]
    cb = tg * (np.asarray(bp, np.float64) @ W1b)             # [256]
    has_cb = bool(np.abs(cb).max() > 1e-12)

    hot = np.empty((128, 10 * PROJ + PROJ + 2 * ID_DIM), BF16)
    hot[:, 0:10 * PROJ] = Wc.reshape(10, 128, PROJ).transpose(1, 0, 2) \
                            .reshape(128, 10 * PROJ).astype(BF16)
    hot[:, 10 * PROJ:11 * PROJ] = np.asarray(W1[:ID_DIM, :]).astype(BF16)
    hot[:, 11 * PROJ:] = np.asarray(W2).reshape(2, 128, ID_DIM) \
                           .transpose(1, 0, 2).reshape(128, 2 * ID_DIM).astype(BF16)
    f32w = np.empty((128, 130), np.float32)
    f32w[:, 0:2] = np.asarray(b1).astype(np.float32).reshape(2, 128).T
    f32w[:, 2:] = np.tile(np.asarray(b2).astype(np.float32).reshape(1, 128), (128, 1))

    shared = {
        "esmbf": np.concatenate(
            [np.asarray(esm_table).astype(BF16), np.zeros((1, ESM_DIM), BF16)], axis=0),
        "idbf": np.asarray(id_table).astype(BF16),
        "hotbf": hot,
        "f32w": f32w,
    }
    tiles = _tile_groups(n_groups)
    per_core = []
    for cc in range(n_cores):
        cols = []
        goff = 0
        for ngrp in tiles:
            s = slice(goff * 128, (goff + ngrp) * 128)
            cols.append(_wrap16(eidx_core[cc, s]))
            cols.append(_wrap16(idid_core[cc, s]))
            goff += ngrp
        pc = {
            "idx16": np.concatenate(cols, axis=1),
            "selbf": sel_core[cc],
        }
        if has_cb:
            pc["mcbbf"] = np.concatenate(
                [mask_core[cc], cb.astype(BF16)]).reshape(1, -1).copy()
        per_core.append(pc)
    return shared, per_core, tok_pos, n_groups, tpg, has_cb


def kernel(idx, gene_idx_to_esm_idx, id_table, esm_table, Wp, bp, gate,
           W1, b1, W2, b2, _trace=False, **_run_kwargs):
    from concourse.bass_utils import run_bass_kernel_spmd

    shared, per_core, tok_pos, n_groups, tpg, has_cb = prepare_host(
        idx, gene_idx_to_esm_idx, id_table, esm_table, Wp, bp, gate, W1, b1, W2, b2)
    bkey = (n_groups, tpg, has_cb)
    if bkey not in _BUILD_CACHE:
        _BUILD_CACHE[bkey] = _build_best(n_groups, tpg, has_cb)
    nc = _BUILD_CACHE[bkey]

    in_maps = [dict(shared, **pc) for pc in per_core]
    res = run_bass_kernel_spmd(nc, in_maps, list(range(N_CORES)), trace=_trace,
                               **_run_kwargs)
    sh = np.asarray(idx).shape
    out = np.empty((NTOK_TOTAL, ID_DIM), np.float32)
    for c in range(N_CORES):
        arr = np.asarray(res.results[c]["out"]).astype(np.float32)  # [128, G*tpg]
        for q in range(n_groups):
            pos = tok_pos[c][q]
            if len(pos):
                out[pos] = arr[:, q * tpg:q * tpg + len(pos)].T
    out = out.reshape(sh[0], sh[1], ID_DIM)
    if _trace:
        return out, res
    return out


# revision 34
# speedup vs baseline: 1.3502x; 1.2181x over previous
"""AugmentedGeneEmbedding kernel for 8 TRN2 NeuronCores (Bass/Tile).

Math (per token t with gene g = idx[t]):
    id_vec  = id_table[g]                                  # [128]
    e       = gene_idx_to_esm_idx[g]
    valid   = (g < N_GENES) & (0 < e < V_ESM)
    seq     = valid ? esm_table[e] @ Wp + bp : 0           # [256]
    h       = concat([id_vec, tanh(gate) * seq])           # [384]
    y       = gelu(h @ W1 + b1) @ W2 + b2                  # [128]

Every factor depends only on the gene, so y[t] = Y[g(t)] for a per-gene
table Y.  The kernel dedups tokens to unique genes and computes Y once
per gene, then expands Y to tokens with one-hot selection matmuls (no
token gather at all):

  Host: fold Wc = tanh(g) * (Wp @ W1_bot)  (and cb = tanh(g) * bp @
      W1_bot, zero for this input); snake-assign unique genes to
      8 cores x G slot-groups of 128 slots each, balancing token counts
      (each group ends up with <=128 genes and <=512 tokens); build
      per-group one-hot SEL[slot, tok] matrices in bf16.  G is the
      smallest group count that fits the unique genes (19 here).
  Phase A (device, per gene tile of <=4 groups): gather esm+id rows
      (transposing SWDGE gather), z = Wc.T@esm + W1_top.T@id
      (+ mask*cb), a = gelu(z + b1), Y_q = a @ W2 + b2 -> SBUF bf16.
  Phase B (device, per 128-slot group, fused into phase A): one PE
      matmul out[feat, tok] = Y_q.T @ SEL_q, DVE copy to bf16, write
      out columns.  Runs immediately after each Y chunk; no DRAM
      round-trip, no phase-B gathers.

Startup is dominated by the fixed NEFF preamble (~6us) and the gpsimd
gather-ucode IRAM load (~6us), so all constant loads are coalesced into
4 HWDGE DMAs (idx / hot weights / f32 biases / SEL) to keep the rings
clear while the ucode loads, and the per-gather index-count register is
hoisted (one MOVE instead of ten).

SWDGE queue plan: gathers rotate through global DMASW sems in
scheduler-emission order; we build once with queue 0, read the emitted
sem rotation, rebuild with queue = sem % 4, verify, else fall back.
"""

import numpy as np
import ml_dtypes

N_CORES = 8
B, K = 32, 2048
N_GENES, ID_DIM, ESM_DIM, PROJ, V_ESM = 20000, 128, 1280, 256, 30000
NTOK_TOTAL = B * K

TPG = 512                         # token columns per slot-group (padded)
DUMMY_GATHER = False              # 16-idx IRAM-warm gather corrupts on HW
QUEUE_PLAN = True                 # rotate gathers across SWDGE queues

BF16 = ml_dtypes.bfloat16

_BUILD_CACHE = {}


def _tile_groups(n_groups):
    """Split n_groups slot-groups into gene tiles: a short 1,2 ramp for the
    earliest possible z start, then 4-group tiles (512-wide moving operands
    keep LDWEIGHTS hidden under the matmul stream)."""
    out = []
    g = 0
    while g < n_groups:
        if not out:
            ngrp = 1
        elif len(out) == 1:
            ngrp = min(2, n_groups - g)
        else:
            ngrp = min(4, n_groups - g)
        out.append(ngrp)
        g += ngrp
    return out


def build_nc(n_groups, tpg, has_cb, queue_plan=None):
    """Per-core Bass program (SPMD: same program on all 8 cores).
    queue_plan maps gather source-index -> SWDGE queue (default all 0).
    Gather source order: esm tile t -> 2t, id tile t -> 2t+1."""
    import concourse.bacc as bacc
    import concourse.mybir as mybir
    import concourse.tile as tile
    from concourse import library_config
    from concourse.tile_rust import add_dep_helper
    from contextlib import ExitStack

    fp32 = mybir.dt.float32
    bf16 = mybir.dt.bfloat16
    i16 = mybir.dt.int16
    AF = mybir.ActivationFunctionType

    TPG = tpg
    ng_cap = n_groups * 128
    tiles = _tile_groups(n_groups)
    n_gt = len(tiles)
    qp = (queue_plan or {}).get
    W16 = ng_cap // 16
    HOT = 10 * PROJ + PROJ + 2 * ID_DIM    # wc | w1t | w2  (bf16 cols)

    nc = bacc.Bacc("TRN2", target_bir_lowering=False, num_swdge_queues=4)

    idx_d = nc.declare_dram_parameter("idx16", [128, 2 * W16], i16, isOutput=False)
    hot_d = nc.declare_dram_parameter("hotbf", [128, HOT], bf16, isOutput=False)
    f32_d = nc.declare_dram_parameter("f32w", [128, 130], fp32, isOutput=False)
    sel_d = nc.declare_dram_parameter("selbf", [128, n_groups * TPG], bf16, isOutput=False)
    esm_d = nc.declare_dram_parameter("esmbf", [V_ESM + 1, ESM_DIM], bf16, isOutput=False)
    id_d = nc.declare_dram_parameter("idbf", [N_GENES, ID_DIM], bf16, isOutput=False)
    if has_cb:
        mcb_d = nc.declare_dram_parameter("mcbbf", [1, ng_cap + PROJ], bf16, isOutput=False)
    # out column q*TPG + j holds token j of slot-group q (features on rows)
    out_d = nc.declare_dram_parameter("out", [128, n_groups * TPG], bf16, isOutput=True)

    with tile.TileContext(nc) as tc, ExitStack() as ctx:
        const = ctx.enter_context(tc.tile_pool(name="const", bufs=1))
        idp = ctx.enter_context(tc.tile_pool(name="idgat", bufs=n_gt))
        gpool = ctx.enter_context(tc.tile_pool(name="gather", bufs=n_gt))
        apool = ctx.enter_context(tc.tile_pool(name="act", bufs=4))
        ypool = ctx.enter_context(tc.tile_pool(name="ygrp", bufs=10))
        opool = ctx.enter_context(tc.tile_pool(name="tokout", bufs=3))
        zps = ctx.enter_context(tc.tile_pool(name="zps", bufs=3, space="PSUM"))
        yps = ctx.enter_context(tc.tile_pool(name="yps", bufs=2, space="PSUM"))
        bps = ctx.enter_context(tc.tile_pool(name="bps", bufs=3, space="PSUM"))

        # Gather ucode library load first, then a throwaway 16-row gather:
        # the first dma_gather on a freshly loaded library pays a ~6us IRAM
        # fetch, so burn it on a dummy while the index DMA is still in
        # flight.  The dummy's index tile is DVE-memset to zero (valid row 0
        # gathers) so it has no DMA dependency at all.
        nc.gpsimd.load_library(library_config.mlp)
        if DUMMY_GATHER:
            dum_idx = const.tile([128, 8], i16)
            nc.vector.memset(dum_idx[:], 0)
            dum_out = const.tile([128, 1, 128], bf16)
            nc.gpsimd.dma_gather(dum_out[:], id_d[:], dum_idx[:], 128,
                                 nc.gpsimd.compute_val(128), ID_DIM,
                                 transpose=True, queue_num=qp(0, 0))

        # Index blob is tile-major ([eidx|idid] per gene tile); tile 0's
        # slice loads as its own tiny DMA so the first gather — and with it
        # the lazy gather-ucode IRAM fetch — unblocks as early as possible.
        idx_sb = const.tile([128, 2 * W16], i16)
        c0 = 16 * tiles[0]
        nc.sync.dma_start(idx_sb[:, 0:c0], idx_d[:, 0:c0])
        nc.sync.dma_start(idx_sb[:, c0:], idx_d[:, c0:])

        # Warm the scalar-engine activation table set containing Gelu during
        # the preamble; otherwise the table load lands mid-stream and blocks
        # the scalar FIFO (and everything downstream) until all DMAs drain.
        f32_sb = const.tile([128, 130], fp32)
        nc.scalar.dma_start(f32_sb[:], f32_d[:])
        b1_sb = f32_sb[:, 0:2]
        b2b_sb = f32_sb[:, 2:130]
        warm = const.tile([128, 1], fp32)
        nc.scalar.activation(warm[:], b1_sb[:, 0:1], AF.Gelu, bias=b1_sb[:, 0:1])

        # Gathers for the whole gene table issued up front; ring backpressure
        # paces them.  esm before id per tile: the z chain consumes esm
        # chunks first, id only at the end.
        nreg = {}
        for gt in sorted(set(tiles)):
            nreg[gt] = nc.gpsimd.compute_val(gt * 128)
        gtiles = []
        itiles = []
        esm_insts = []
        goff = 0
        for t, ngrp in enumerate(tiles):
            gn = ngrp * 128
            ic = goff * 16                 # tile-major idx blob
            gtile = gpool.tile([128, 10, gn], bf16, tag="G", name=f"G{t}")
            gi = nc.gpsimd.dma_gather(gtile[:], esm_d[:],
                                      idx_sb[:, ic:ic + gn // 16], gn, nreg[ngrp],
                                      ESM_DIM, transpose=True,
                                      queue_num=qp(1 + 2 * t, 0))
            esm_insts.append(gi)
            gtiles.append(gtile)
            itile = idp.tile([128, 1, gn], bf16, tag="I", name=f"I{t}")
            nc.gpsimd.dma_gather(itile[:], id_d[:],
                                 idx_sb[:, ic + gn // 16:ic + gn // 8], gn, nreg[ngrp],
                                 ID_DIM, transpose=True,
                                 queue_num=qp(2 + 2 * t, 0))
            itiles.append(itile)
            goff += ngrp

        # Weight loads after gather issuance in program order.
        hot_sb = const.tile([128, HOT], bf16)
        nc.sync.dma_start(hot_sb[:], hot_d[:])
        wc_sb = hot_sb[:, 0:10 * PROJ]               # [(c, f)] flat
        w1t_sb = hot_sb[:, 10 * PROJ:10 * PROJ + PROJ]
        w2_sb = hot_sb[:, 11 * PROJ:11 * PROJ + 2 * ID_DIM]
        if has_cb:
            mcb_sb = const.tile([1, ng_cap + PROJ], bf16)
            nc.scalar.dma_start(mcb_sb[:], mcb_d[:])
            mask_sb = mcb_sb[:, 0:ng_cap]
            cb_sb = mcb_sb[:, ng_cap:]
        # SEL loads up front: the B matmuls sit in the PE FIFO, so SEL data
        # must never arrive after the gather stream (SWDGE packets starve
        # HWDGE traffic once the gathers saturate the engines).  Loading SEL
        # in the same early window as the weights costs the gather start a
        # little, which the small first tiles absorb.
        sel_sb = const.tile([128, n_groups * TPG], bf16)
        nc.scalar.dma_start(sel_sb[:], sel_d[:])

        # ---------- fused phase A (per-gene Y) + phase B (token expand) ----
        # Phase B for tile t-1 is emitted AFTER tile t's z/y matmuls: the PE
        # queue is strict FIFO, so a B matmul whose SEL slice hasn't landed
        # yet must not sit in front of the next tile's (data-ready) z chain.
        def emit_B(pend):
            goff_, ngrp_, yqs = pend
            osb = opool.tile([128, ngrp_, TPG], bf16, tag="o", name=f"o{goff_}")
            for qq in range(ngrp_):
                q = goff_ + qq
                bb = bps.tile([128, TPG], fp32, tag="b")
                nc.tensor.matmul(bb[:], yqs[qq][:],
                                 sel_sb[:, q * TPG:(q + 1) * TPG],
                                 start=True, stop=True)
                nc.vector.tensor_copy(osb[:, qq, :], bb[:])
            nc.sync.dma_start(out_d[:, goff_ * TPG:(goff_ + ngrp_) * TPG],
                              osb[:].rearrange("p a b -> p (a b)"))

        pending = None
        goff = 0
        for t, ngrp in enumerate(tiles):
            gn = ngrp * 128
            gtile = gtiles[t]
            a_tiles = []
            for h in range(2):
                hs = slice(h * 128, (h + 1) * 128)
                zp = zps.tile([128, gn], fp32, tag="z", name=f"z{t}_{h}")
                for c in range(10):
                    nc.tensor.matmul(zp[:], wc_sb[:, c * PROJ + h * 128:
                                                  c * PROJ + h * 128 + 128],
                                     gtile[:, c, :], start=c == 0, stop=False)
                # id contribution late: each chain starts on esm data alone,
                # giving the (latency-bound) id gathers extra slack
                nc.tensor.matmul(zp[:], w1t_sb[:, hs], itiles[t][:, 0, :],
                                 start=False, stop=not has_cb)
                if has_cb:
                    nc.tensor.matmul(zp[:], cb_sb[0:1, hs],
                                     mask_sb[0:1, goff * 128:goff * 128 + gn],
                                     start=False, stop=True)
                at = apool.tile([128, gn], bf16, tag="a", name=f"a{t}_{h}")
                nc.scalar.activation(at[:], zp[:], AF.Gelu, bias=b1_sb[:, h:h + 1])
                a_tiles.append(at)
            yqs = []
            for qq in range(ngrp):
                qs = slice(qq * 128, (qq + 1) * 128)
                yp = yps.tile([128, 128], fp32, tag="yp")
                nc.tensor.matmul(yp[:], a_tiles[0][:, qs], w2_sb[:, 0:ID_DIM],
                                 start=True, stop=False)
                nc.tensor.matmul(yp[:], a_tiles[1][:, qs], w2_sb[:, ID_DIM:],
                                 start=False, stop=True)
                yq = ypool.tile([128, 128], bf16, tag="y")
                nc.vector.tensor_add(yq[:], yp[:], b2b_sb[:])
                yqs.append(yq)
            if pending is not None:
                emit_B(pending)
            pending = (goff, ngrp, yqs)
            goff += ngrp
        emit_B(pending)

    nc.compile()
    return nc


def _gather_emission(nc):
    """(num_idxs, elem_size, transpose, queue, sem_idx) per InstDMAGatherAnt
    in emission order."""
    import re
    out = []
    for i in nc.all_instructions():
        if type(i).__name__ != "InstDMAGatherAnt":
            continue
        sem = None
        if i.sync_info is not None:
            for u in i.sync_info.on_update:
                m = re.search(r"DMASW(\d+)_", str(u))
                if m:
                    sem = int(m.group(1))
        out.append((int(i.num_idxs), int(i.elem_size), bool(i.transpose),
                    int(i.queue_num), sem))
    return out


def _plan_queues(nc, n_groups):
    """Map gather source-index -> queue from the pass-1 sem rotation."""
    em = _gather_emission(nc)
    src = [(0, (128, ID_DIM, True))] if DUMMY_GATHER else []
    for t, ngrp in enumerate(_tile_groups(n_groups)):
        src.append((1 + 2 * t, (ngrp * 128, ESM_DIM, True)))
        src.append((2 + 2 * t, (ngrp * 128, ID_DIM, True)))
    if len(em) != len(src):
        return None
    from collections import defaultdict, deque
    pools = defaultdict(deque)
    for (ni, es, tr, q, sem) in em:
        if sem is None:
            return None
        pools[(ni, es, tr)].append(sem)
    plan = {}
    for si, sig in src:
        if not pools[sig]:
            return None
        plan[si] = pools[sig].popleft() % 4
    return plan


def _queues_consistent(nc):
    sems = {}
    for (ni, es, tr, q, sem) in _gather_emission(nc):
        if sem is None:
            return False
        if sems.setdefault(sem, q) != q:
            return False
    return True


def _build_best(n_groups, tpg, has_cb):
    nc0 = build_nc(n_groups, tpg, has_cb, None)
    if not QUEUE_PLAN:
        return nc0
    try:
        plan = _plan_queues(nc0, n_groups)
        if plan and any(q != 0 for q in plan.values()):
            nc1 = build_nc(n_groups, tpg, has_cb, plan)
            if _queues_consistent(nc1):
                return nc1
    except Exception:
        pass
    return nc0


def _wrap16(a16):
    """int16 [n] -> [128, n//16]: logical index i at [i % 16 (+16k), i // 16]."""
    w = a16.reshape(-1, 16).T
    return np.tile(w, (8, 1)).copy()


def _assign_bins(cnt, n_cores, n_groups):
    """Snake-assign genes (by count desc) to n_cores*n_groups bins.
    Returns (bin_of, ok): ok=False if any bin exceeds 128 genes or TPG
    tokens."""
    U = len(cnt)
    NB = n_cores * n_groups
    order = np.argsort(-cnt, kind="stable")
    k = np.arange(U)
    rnd = k // NB
    c = k % NB
    bin_snake = np.where(rnd % 2 == 0, c, NB - 1 - c)
    bin_of = np.empty(U, np.int64)
    bin_of[order] = bin_snake
    gcnt = np.bincount(bin_of, minlength=NB)
    tcnt = np.bincount(bin_of, weights=cnt, minlength=NB)
    return bin_of, bool(gcnt.max() <= 128 and tcnt.max() <= TPG)


def prepare_host(idx, gene_idx_to_esm_idx, id_table, esm_table, Wp, bp, gate,
                 W1, b1, W2, b2, n_cores=N_CORES):
    """Index prep, weight folding, dtype/layout marshalling.

    Returns (shared, per_core, tok_pos, n_groups, tpg, has_cb); tok_pos[c][q]
    are the original flat token positions in slot-group q of core c, in
    SEL column order."""
    idx_flat = np.asarray(idx).reshape(-1).astype(np.int64)
    gmap = np.asarray(gene_idx_to_esm_idx).astype(np.int64)
    g_clip = np.clip(idx_flat, 0, N_GENES - 1)
    oob = (idx_flat < 0) | (idx_flat >= N_GENES)
    # key encodes (id row, forced-invalid) so OOB tokens get mask=0 entries
    key = np.where(oob, g_clip + N_GENES, g_clip)
    uniq, inv = np.unique(key, return_inverse=True)
    U = len(uniq)
    cnt = np.bincount(inv, minlength=U)

    n_groups = -(-U // (128 * n_cores))
    bin_of, ok = _assign_bins(cnt, n_cores, n_groups)
    while not ok:
        n_groups += 1
        bin_of, ok = _assign_bins(cnt, n_cores, n_groups)
    NB = n_cores * n_groups
    ng_cap = n_groups * 128
    core_of = bin_of % n_cores
    grp_of = bin_of // n_cores
    # within each bin, order genes by key value (ascending table reads)
    rank_of = np.empty(U, np.int64)
    for b in range(NB):
        m = np.nonzero(bin_of == b)[0]        # ascending key order
        rank_of[m] = np.arange(len(m))
    slot_of = grp_of * 128 + rank_of

    urow = np.where(uniq >= N_GENES, uniq - N_GENES, uniq)   # id-table row
    ue = gmap[np.clip(urow, 0, N_GENES - 1)]
    uvalid = (uniq < N_GENES) & (ue > 0) & (ue < V_ESM)
    ueidx = np.where(uvalid, ue, V_ESM)                      # row V_ESM is zero pad

    eidx_core = np.full((n_cores, ng_cap), V_ESM, np.int16)
    idid_core = np.zeros((n_cores, ng_cap), np.int16)
    mask_core = np.zeros((n_cores, ng_cap), BF16)
    eidx_core[core_of, slot_of] = ueidx.astype(np.int16)
    idid_core[core_of, slot_of] = urow.astype(np.int16)
    mask_core[core_of, slot_of] = uvalid.astype(BF16)

    # tokens -> SEL one-hots: column j of (core, group) = j-th token of that
    # bin in flat order.  SEL stored partition-major: sel[p, q*TPG+j].
    tok_bin = bin_of[inv]
    tok_rank = rank_of[inv]
    bin_sort = np.argsort(tok_bin, kind="stable")  # flat positions by bin
    bcnt = np.bincount(tok_bin, minlength=NB)
    boff = np.concatenate([[0], np.cumsum(bcnt)])
    tpg = min(TPG, int(-(-int(bcnt.max()) // 64) * 64))   # padded col count
    sel_core = np.zeros((n_cores, 128, n_groups * tpg), BF16)
    tok_pos = [[None] * n_groups for _ in range(n_cores)]
    for b in range(NB):
        pos = bin_sort[boff[b]:boff[b + 1]]
        cc, q = b % n_cores, b // n_cores
        tok_pos[cc][q] = pos
        sel_core[cc, tok_rank[pos], q * tpg + np.arange(len(pos))] = 1

    # host weight folding
    tg = np.tanh(float(np.asarray(gate).reshape(-1)[0]))
    Wp64 = np.asarray(Wp, np.float64)
    W1b = np.asarray(W1, np.float64)[ID_DIM:, :]
    Wc = tg * (Wp64 @ W1b)                                   # [1280, 256]
    cb = tg * (np.asarray(bp, np.float64) @ W1b)             # [256]
    has_cb = bool(np.abs(cb).max() > 1e-12)

    hot = np.empty((128, 10 * PROJ + PROJ + 2 * ID_DIM), BF16)
    hot[:, 0:10 * PROJ] = Wc.reshape(10, 128, PROJ).transpose(1, 0, 2) \
                            .reshape(128, 10 * PROJ).astype(BF16)
    hot[:, 10 * PROJ:11 * PROJ] = np.asarray(W1[:ID_DIM, :]).astype(BF16)
    hot[:, 11 * PROJ:] = np.asarray(W2).reshape(2, 128, ID_DIM) \
                           .transpose(1, 0, 2).reshape(128, 2 * ID_DIM).astype(BF16)
    f32w = np.empty((128, 130), np.float32)
    f32w[:, 0:2] = np.asarray(b1).astype(np.float32).reshape(2, 128).T
    f32w[:, 2:] = np.tile(np.asarray(b2).astype(np.float32).reshape(1, 128), (128, 1))

    shared = {
        "esmbf": np.concatenate(
            [np.asarray(esm_table).astype(BF16), np.zeros((1, ESM_DIM), BF16)], axis=0),
        "idbf": np.asarray(id_table).astype(BF16),
        "hotbf": hot,
        "f32w": f32w,
    }
    tiles = _tile_groups(n_groups)
    per_core = []
    for cc in range(n_cores):
        cols = []
        goff = 0
        for ngrp in tiles:
            s = slice(goff * 128, (goff + ngrp) * 128)
            cols.append(_wrap16(eidx_core[cc, s]))
            cols.append(_wrap16(idid_core[cc, s]))
            goff += ngrp
        pc = {
            "idx16": np.concatenate(cols, axis=1),
            "selbf": sel_core[cc],
        }
        if has_cb:
            pc["mcbbf"] = np.concatenate(
                [mask_core[cc], cb.astype(BF16)]).reshape(1, -1).copy()
        per_core.append(pc)
    return shared, per_core, tok_pos, n_groups, tpg, has_cb


def kernel(idx, gene_idx_to_esm_idx, id_table, esm_table, Wp, bp, gate,
           W1, b1, W2, b2, _trace=False, **_run_kwargs):
    from concourse.bass_utils import run_bass_kernel_spmd

    shared, per_core, tok_pos, n_groups, tpg, has_cb = prepare_host(
        idx, gene_idx_to_esm_idx, id_table, esm_table, Wp, bp, gate, W1, b1, W2, b2)
    bkey = (n_groups, tpg, has_cb)
    if bkey not in _BUILD_CACHE:
        _BUILD_CACHE[bkey] = _build_best(n_groups, tpg, has_cb)
    nc = _BUILD_CACHE[bkey]

    in_maps = [dict(shared, **pc) for pc in per_core]
    res = run_bass_kernel_spmd(nc, in_maps, list(range(N_CORES)), trace=_trace,
                               **_run_kwargs)
    sh = np.asarray(idx).shape
    out = np.empty((NTOK_TOTAL, ID_DIM), np.float32)
    for c in range(N_CORES):
        arr = np.asarray(res.results[c]["out"]).astype(np.float32)  # [128, G*tpg]
        for q in range(n_groups):
            pos = tok_pos[c][q]
            if len(pos):
                out[pos] = arr[:, q * tpg:q * tpg + len(pos)].T
    out = out.reshape(sh[0], sh[1], ID_DIM)
    if _trace:
        return out, res
    return out
